# revision 40
# baseline (speedup 1.0000x reference)
"""AudioAttNet Trainium2 kernel, v8.

Computation per batch element b (65536 total):
  x[29, 8] -> conv1d(29->16, k=3) + lrelu -> conv(16->8) + lrelu
           -> conv(8->4) + lrelu -> conv(4->128) + lrelu = y [128, 8]
  logits = y^T @ wl^T ; attn = softmax(logits, axis=seq)
  out = sum_seq(y^T * attn)  = [128]

Mapping: pure data parallel over batch across 8 cores (8192/core).
Host prep: x is converted to f16, transposed to [(c,s)=232(+ones row), B]
and padded with a constant-one row so conv1's bias rides the matmul.
All conv biases are folded into the matmuls (ones-rows); conv4 runs as
eight K=33 matmuls (32 taps + bias row from a persistent ones row in y3).

v8 structure (~242us vs v7's 256us; steady state is scalar-engine
bound at ~19.6us/1024-chunk of PSUM evacuations + exp):
 - Two depth-1 PSUM rings of [128, 4, 512] f32 (4 banks each): conv4
   s-pair groups + c123 in psC, linear s-pairs in psL, emitted
   interleaved so each ring's FD=2048 scalar evacuation (~1.97us,
   vs 2x1.2us at FD=1024) hides behind the other ring's matmuls.
 - Pipeline: iteration i emits conv4(i) x linear(i-1) interleaved |
   conv1/2/3(i+1) | tail(i-2); every stage consumes data produced >= 1
   full iteration earlier.  The serial c1->c2->c3 PE<->scalar chain
   sits at the iteration end where its stalls overlap the tail phase.
 - Epilogue: last chunk's linear groups interleave with the
   second-to-last tail's pieces; the final tail runs as two
   batch-halves so its vector/gpsimd chain links pipeline (drain
   ~30us -> ~20us).
 - PE heater at kernel start (HAM clock-gate: PE defaults to 1.2 GHz;
   ~3.4us of dense matmul activity raises it to 2.4 GHz).  In steady
   state the PE oscillates warm/cold once per iteration - the ~2us
   ring round-trip bubble at each iteration boundary re-throttles it.

Engine balance per 1024-batch chunk (measured): scalar 19.6us (4 conv4
evacs + 4 exps at FD=2048 + c1/c2/c3), vector ~18us (numerator product
+ tree folds + reciprocal; several f16 tensor_tensor ops run at 1x
instead of 2x - flattening APs or splitting tiles does NOT fix it),
gpsimd 13us (SBUF f16 adds at ~2ns/elem), tensor ~18us cold / ~9 warm.

Tried and rejected on hardware: gpsimd scalar_tensor_tensor (illegal -
TensorScalarPtr not a Pool-engine opcode), single shared PSUM ring
(serializes PE behind the saturated vector queue when poly-exp holds a
slot), DVE poly-exp offload (vector has no slack), b-halved steady
tails (strided APs lose the DVE 2x mode), flattened 2D fold APs
(slower), interspersing c123 between matmul groups (in-order PE queue
stalls), c123 psum in the linear ring (delays exp ring), BC=512/2048
(282/253us), LDWEIGHTS-filler for HAM (no effect), heater count 0-40
(neutral in steady state).  gpsimd cannot access PSUM on TRN2; SWDGE
accumulate-DMA faults the exec unit (v7).
"""

import os
import numpy as np
from contextlib import ExitStack

import concourse.bass as bass
from concourse import bacc
from concourse import mybir
from concourse.bass_utils import run_bass_kernel_spmd

F16 = mybir.dt.float16
F32 = mybir.dt.float32
AF = mybir.ActivationFunctionType
ALU = mybir.AluOpType

B, C, S = 65536, 29, 8
NCORES = 8
BPC = B // NCORES            # batches per core
BC = int(os.environ.get("CC_BC", "1024"))  # batches per chunk
NCHUNK = BPC // BC
NT = BC // 512               # 512-wide t-tiles per chunk
GSP = 4 // NT                # conv4/linear s-slices per 4-slot psum tile
NGRP = S // GSP              # matmul groups per chunk
TLAG = 2 if NCHUNK > 4 else 1  # tail lag (chunks)
CS = C * S                   # 232
XROWS = CS + 1               # +1 ones row for the conv1 bias
NEG = 0.02

# conv4 s-pairs whose evac runs as a 2-op DVE sequence instead of scalar
_SPLIT = set(int(c) for c in os.environ.get("CC_SPLIT", ""))
# linear slices whose exp runs on the vector engine as a polynomial
_POLY = set(int(c) for c in os.environ.get("CC_POLY", ""))
# engine for the c1 evacuation: "scalar" or "dve" (c2/c3 need the bias
# AP, which the DVE path lacks, so they always go scalar)
_C123 = os.environ.get("CC_C123", "scalar")
_HEAT = int(os.environ.get("CC_HEAT", "16"))
# dummy LDWEIGHTS sprinkled between matmul groups: PSUM-free PE
# activity that keeps the HAM clock-gate monitor fed during gaps
_FILL = int(os.environ.get("CC_FILL", "0"))


def _build_nc():
    nc = bacc.Bacc()

    x_in = nc.declare_dram_parameter("xt", [XROWS, BPC], F16, isOutput=False)
    w1a_d = nc.declare_dram_parameter("w1a", [128, 128], F16, isOutput=False)
    w1b_d = nc.declare_dram_parameter("w1b", [105, 128], F16, isOutput=False)
    w2_d = nc.declare_dram_parameter("w2e", [128, 64], F16, isOutput=False)
    w3_d = nc.declare_dram_parameter("w3e", [64, 32], F16, isOutput=False)
    w4_d = nc.declare_dram_parameter("w4s", [33, 8 * 128], F16, isOutput=False)
    wl_d = nc.declare_dram_parameter("wlt", [128, 128], F16, isOutput=False)
    b2_d = nc.declare_dram_parameter("b2v", [64, 1], F32, isOutput=False)
    b3_d = nc.declare_dram_parameter("b3v", [32, 1], F32, isOutput=False)
    out_d = nc.declare_dram_parameter("out", [128, BPC], F16, isOutput=True)

    from concourse.tile import TileContext

    with TileContext(nc) as tc, ExitStack() as ctx:
        consts = ctx.enter_context(tc.tile_pool(name="consts", bufs=1))
        w1a = consts.tile_from(w1a_d[:])
        w1b = consts.tile_from(w1b_d[:])
        w2e = consts.tile_from(w2_d[:])
        w3e = consts.tile_from(w3_d[:])
        w4s_flat = consts.tile_from(w4_d[:])
        w4s = w4s_flat[:].rearrange("p (s d) -> p s d", s=8)
        wlt = consts.tile_from(wl_d[:])
        b2v = consts.tile_from(b2_d[:])
        b3v = consts.tile_from(b3_d[:])
        alpha_v = consts.tile([128, 1], F32)
        nc.vector.memset(alpha_v[:], NEG)
        # warm the Exp/Prelu activation table before the first conv
        warm = consts.tile([1, 1], F16)
        nc.scalar.activation(warm[:], alpha_v[0:1, :], AF.Exp)

        # persistent, manually double-buffered tiles (ones rows set once)
        y3_bufs = [consts.tile([33, BC], F16, name=f"y3_{i}") for i in range(2)]
        for t in y3_bufs:
            nc.vector.memset(t[32:33, :], 1.0)

        io = ctx.enter_context(tc.tile_pool(name="io", bufs=2))
        acts = ctx.enter_context(tc.tile_pool(name="acts", bufs=2))
        big = ctx.enter_context(tc.tile_pool(name="bigsb", bufs=4))
        tail = ctx.enter_context(tc.tile_pool(name="tailp", bufs=1))
        # two independent PSUM rings of 1 x [128, 4, 512] f32 (4 banks
        # each): conv4 s-pairs + c123 rotate through psC, linear s-pairs
        # through psL.  conv4 and linear groups are emitted interleaved,
        # so each ring's evacuation (FD=2048) hides behind the other
        # ring's matmuls.
        psC = ctx.enter_context(tc.tile_pool(name="psC", bufs=1, space="PSUM"))
        psL = ctx.enter_context(tc.tile_pool(name="psL", bufs=1, space="PSUM"))

        # ---- PE heater: un-throttle the HAM clock gate before conv1 ----
        if _HEAT:
            hw = consts.tile([128, 128], F16, name="heatw")
            hr = consts.tile([128, 512], F16, name="heatr")
            nc.vector.memset(hw[:], 0.0)
            nc.vector.memset(hr[:], 0.0)
            ph = psC.tile([128, 4, 512], F32, tag="psC", name="pheat")
            for _ in range(_HEAT):
                nc.tensor.matmul(ph[:, 0], hw[:], hr[:], start=True, stop=True)

        def evac_prelu(eng, dst, src, nslice):
            """dst = lrelu(src) (bias already in src). src is an f32 psum
            AP; dst a matching f16 AP."""
            if eng == "scalar":
                nc.scalar.activation(dst, src, AF.Prelu,
                                     alpha=alpha_v[0:src.shape[0], :])
            else:  # dve: t = 0.02*psum; dst = max(t, psum)
                a = src.shape[1]
                tmp = tail.tile([128, 2 * BC], F16, tag="tmps",
                                name=f"tmp{nslice}", bufs=3)
                tv = tmp[:src.shape[0], 0:a * 512].rearrange(
                    "p (a b) -> p a b", a=a)
                nc.vector.tensor_scalar(tv, src, NEG, None, ALU.mult)
                nc.vector.tensor_max(dst, tv, src)

        def emit_exp(k, dst, pl):
            """dst = exp(pl). Slices in _POLY run on the vector engine as
            (0.5(l/2+1)^2+0.5)^2 (|l|<0.4 -> rel err < 2e-3), relieving the
            scalar engine; the rest use the Exp table.  dst is [p, a, 512]
            with a in {2, 4}; pl the matching psum view."""
            if k in _POLY:
                a = dst.shape[1]
                q = tail.tile([128, 2 * BC], F16, tag="tmps", name=f"q{k}",
                              bufs=3)
                qv = q[:, 0:a * 512].rearrange("p (a b) -> p a b", a=a)
                nc.vector.tensor_scalar(qv, pl, 0.5, 1.0, ALU.mult,
                                        ALU.add)
                nc.vector.tensor_mul(qv, qv, qv)
                nc.vector.tensor_scalar(qv, qv, 0.5, 0.5, ALU.mult, ALU.add)
                nc.vector.tensor_mul(dst, qv, qv)
            else:
                nc.scalar.activation(dst, pl, AF.Exp)

        def load(ch):
            xt1 = io.tile([128, BC], F16, tag="xt1", name="xt1")
            xt2 = io.tile([105, BC], F16, tag="xt2", name="xt2")
            sl = slice(ch * BC, (ch + 1) * BC)
            nc.sync.dma_start(out=xt1[:], in_=x_in[0:128, sl])
            nc.sync.dma_start(out=xt2[:], in_=x_in[128:XROWS, sl])
            return xt1, xt2

        def conv1(ch, xt):
            xt1, xt2 = xt
            # bias via xt2 ones row; w1a then w1b so each stationary is
            # loaded once
            y1 = acts.tile([128, BC], F16, tag="y1", name="y1")
            p1 = psC.tile([128, 4, 512], F32, tag="psC", name="p1")
            for t in range(NT):
                nc.tensor.matmul(p1[:, t], w1a[:], xt1[:, t * 512:(t + 1) * 512],
                                 start=True, stop=False)
            for t in range(NT):
                nc.tensor.matmul(p1[:, t], w1b[:], xt2[:, t * 512:(t + 1) * 512],
                                 start=False, stop=True)
            evac_prelu(_C123, y1[:].rearrange("p (a b) -> p a b", a=NT),
                       p1[:, 0:NT], "c1")
            return y1

        def conv2(ch, y1):
            y2 = acts.tile([64, BC], F16, tag="y2", name="y2")
            p2 = psC.tile([64, 4, 512], F32, tag="psC", name="p2")
            for t in range(NT):
                nc.tensor.matmul(p2[:, t], w2e[:], y1[:, t * 512:(t + 1) * 512],
                                 start=True, stop=True)
            nc.scalar.activation(y2[:].rearrange("p (a b) -> p a b", a=NT),
                                 p2[:, 0:NT], AF.Prelu, bias=b2v[:],
                                 alpha=alpha_v[0:64, :])
            return y2

        def conv3(ch, y2):
            y3 = y3_bufs[ch % 2]
            p3 = psC.tile([32, 4, 512], F32, tag="psC", name="p3")
            for t in range(NT):
                nc.tensor.matmul(p3[:, t], w3e[:], y2[:, t * 512:(t + 1) * 512],
                                 start=True, stop=True)
            nc.scalar.activation(y3[0:32, :].rearrange("p (a b) -> p a b", a=NT),
                                 p3[:, 0:NT], AF.Prelu, bias=b3v[:],
                                 alpha=alpha_v[0:32, :])

        def c4_group(sp, y3, yy):
            p4 = psC.tile([128, 4, 512], F32, tag="psC", name=f"p4_{sp}")
            for j in range(GSP):
                s = GSP * sp + j
                for t in range(NT):
                    nc.tensor.matmul(p4[:, NT * j + t], w4s[:, s, :],
                                     y3[:, t * 512:(t + 1) * 512],
                                     start=True, stop=True)
            dst = yy[:, GSP * sp:GSP * sp + GSP, :].rearrange(
                "p s (t b) -> p s t b", t=NT)
            evac_prelu("dve" if sp in _SPLIT else "scalar", dst,
                       p4[:].rearrange("p (s t) b -> p s t b", s=GSP),
                       f"c4_{sp}")

        def lin_group(sp, ye, defer=None):
            """Linear matmuls for s-pair sp.  With defer=list, the exp
            evacuation is not emitted; a closure is appended instead so
            the caller can slot it into scalar-engine wait gaps (e.g.
            between the c1/c2/c3 evacuations, which stall on their own
            matmuls)."""
            yy, eep = ye
            pl = psL.tile([128, 4, 512], F32, tag="psL", name=f"pl_{sp}")
            for j in range(GSP):
                s = GSP * sp + j
                for t in range(NT):
                    nc.tensor.matmul(pl[:, NT * j + t], wlt[:],
                                     yy[:, s, t * 512:(t + 1) * 512],
                                     start=True, stop=True)
            ss = [GSP * sp + j for j in range(GSP)]

            def _emit():
                if all((s in _POLY) == (ss[0] in _POLY) for s in ss):
                    dst = eep[:, ss[0]:ss[-1] + 1, :].rearrange(
                        "p s (t b) -> p s t b", t=NT)
                    emit_exp(ss[0], dst,
                             pl[:].rearrange("p (s t) b -> p s t b", s=GSP))
                else:
                    for j, s in enumerate(ss):
                        emit_exp(s, eep[:, s].rearrange("p (a b) -> p a b",
                                                        a=NT),
                                 pl[:, NT * j:NT * j + NT])

            if defer is None:
                _emit()
            else:
                defer.append(_emit)

        def filler():
            for _ in range(_FILL):
                nc.tensor.ldweights(hw[:])

        def conv4_lin(ch, ye_prev, xt_next, ye_tail=None, ye_back=None):
            """conv4 s-pair groups of chunk ch interleaved with linear
            s-pair groups of chunk ch-1 (each depth-1 psum ring's
            evacuation hides behind the other ring's matmuls), then the
            conv1/2/3 chain of chunk ch+1 (at the end, where its serial
            PE<->scalar chain overlaps the tail phase)."""
            y3 = y3_bufs[ch % 2]
            # the deferred back half of an older chunk is emitted before
            # this chunk's yy/ee allocations so the recycled buffers'
            # last readers precede the new writes in program order
            if ye_back:
                tail_back(ch - TLAG - 1, ye_back)
            # s-stride padded to BC+64 so tree-fold operands are not an
            # exact 8KB apart (SBUF port-conflict heuristic)
            yy = big.tile([128, S, BC + 64], F16, tag="yy",
                          name="yy")[:, :, 0:BC]
            ee = big.tile([128, S, BC + 64], F16, tag="ee",
                          name="ee")[:, :, 0:BC]
            deferred = []
            for sp in range(NGRP):
                # lin group first: its inputs (ye_prev, its psum slot) have
                # been ready since last iteration, so the PE always has
                # work while c4_group(0) waits for the previous
                # iteration's final c3 evacuation to free the psC ring.
                if ye_prev is not None:
                    lin_group(sp, ye_prev,
                              defer=deferred if sp >= NGRP - 2 else None)
                c4_group(sp, y3, yy)
                filler()
            if xt_next is not None:
                # the deferred exps of the last two lin groups slot into
                # the scalar gaps where the c1->c2->c3 chain waits on its
                # own matmuls
                y1 = conv1(ch + 1, xt_next)
                if deferred:
                    deferred.pop(0)()
                y2 = conv2(ch + 1, y1)
                if deferred:
                    deferred.pop(0)()
                conv3(ch + 1, y2)
            for fn in deferred:
                fn()
            if ye_tail:
                for _ in tail_front(ch - TLAG, ye_tail):
                    pass
            return yy, ee

        def lin_only(ch, ye):
            for sp in range(NGRP):
                lin_group(sp, ye)
                filler()

        def tail_front(ch, ye, b=slice(0, BC)):
            """Tail front half: numerator product + s-tree folds (vector/
            gpsimd split).  Generator of 3 pieces for interleaving."""
            yy, ee = ye
            nc.vector.tensor_mul(yy[:, :, b], yy[:, :, b], ee[:, :, b])
            yield
            # L1 (full-width slices are contiguous runs)
            nc.vector.tensor_add(yy[:, 0:4, b], yy[:, 0:4, b], yy[:, 4:8, b])
            nc.gpsimd.tensor_add(ee[:, 0:2, b], ee[:, 0:2, b], ee[:, 4:6, b])
            yield
            nc.vector.tensor_add(ee[:, 2:4, b], ee[:, 2:4, b], ee[:, 6:8, b])
            nc.vector.tensor_add(yy[:, 0:2, b], yy[:, 0:2, b], yy[:, 2:4, b])
            nc.gpsimd.tensor_add(ee[:, 0:2, b], ee[:, 0:2, b], ee[:, 2:4, b])
            nc.gpsimd.tensor_add(yy[:, 0, b], yy[:, 0, b], yy[:, 1, b])
            yield

        def tail_back(ch, ye, b=slice(0, BC)):
            """Tail back half: den sum, reciprocal, final mul + store.
            These wait on the gpsimd fold links, so they are emitted one
            iteration after the front — by then the gpsimd work is done
            and the in-order vector queue never head-of-line blocks the
            next chunk's product behind a gpsimd wait."""
            yy, ee = ye
            dd = tail.tile([128, BC], F32, tag="dd", name="dd")
            nc.vector.tensor_add(dd[:, b], ee[:, 0, b], ee[:, 1, b])
            rr = tail.tile([128, BC], F32, tag="rr", name="rr")
            nc.vector.reciprocal_approx_fast(rr[:, b], dd[:, b])
            oo = tail.tile([128, BC], F16, tag="oo", name="oo")
            nc.vector.tensor_mul(oo[:, b], yy[:, 0, b], rr[:, b])
            nc.sync.dma_start(out=out_d[:, ch * BC + b.start:ch * BC + b.stop],
                              in_=oo[:, b])

        def tail_chunk(ch, ye):
            for _ in tail_front(ch, ye):
                pass
            tail_back(ch, ye)

        # ---- pipeline: iteration i emits conv4(i) x linear(i-1)
        # interleaved | conv123(i+1) | tail(i-2); every stage consumes
        # data produced a full iteration earlier.  conv123 sits after the
        # interleave so a conv2 matmul waiting on the c1 evacuation never
        # blocks older PE work (the PE queue is in-order). ----
        repeat = int(os.environ.get("CC_REPEAT", "1"))
        for _rep in range(repeat):
            xts = {0: load(0)}
            if NCHUNK > 1:
                xts[1] = load(1)
            y1_0 = conv1(0, xts[0])
            y2_0 = conv2(0, y1_0)
            conv3(0, y2_0)
            xts.pop(0)
            yes = {}
            for i in range(NCHUNK):
                if i + 2 < NCHUNK:
                    xts[i + 2] = load(i + 2)
                yes[i] = conv4_lin(i, yes.get(i - 1),
                                   xts.pop(i + 1, None),
                                   yes.get(i - TLAG),
                                   yes.pop(i - TLAG - 1, None))
            # epilogue: the loop has emitted fronts for chunks <= N-3 and
            # backs for chunks <= N-4.  Pending: back(N-3), front+back of
            # N-2 and N-1.  Interleave the last chunk's linear groups with
            # back(N-3) and front(N-2); drain the final tails as
            # batch-halves so their vector/gpsimd chain links pipeline.
            tp = (iter(tail_front(NCHUNK - 2, yes[NCHUNK - 2]))
                  if (NCHUNK - 2) in yes else None)
            for sp in range(NGRP):
                lin_group(sp, yes[NCHUNK - 1])
                if sp == 0 and (NCHUNK - 3) in yes:
                    tail_back(NCHUNK - 3, yes.pop(NCHUNK - 3))
                elif tp is not None:
                    next(tp, None)
            if tp is not None:
                for _ in tp:
                    pass
            if (NCHUNK - 2) in yes:
                tail_back(NCHUNK - 2, yes.pop(NCHUNK - 2))
            ye_last = yes.pop(NCHUNK - 1)
            ha = iter(tail_front(NCHUNK - 1, ye_last, slice(0, BC // 2)))
            hb = iter(tail_front(NCHUNK - 1, ye_last, slice(BC // 2, BC)))
            next(ha, None)
            while True:
                a = next(ha, 0)
                b = next(hb, 0)
                if a == 0 and b == 0:
                    break
            tail_back(NCHUNK - 1, ye_last, slice(0, BC // 2))
            tail_back(NCHUNK - 1, ye_last, slice(BC // 2, BC))

    nc.compile()
    return nc


def _host_weights(w1, b1, w2, b2, w3, b3, w4, b4, wl):
    # conv-as-matmul weights; rows are (cin, s_in) flattened, cols (cout,
    # s_out) flattened; zero where the kernel tap falls outside.
    def eff(wc, cin, cout):
        m = np.zeros((cin * S, cout * S), np.float32)
        for co in range(cout):
            for ci in range(cin):
                for k in range(3):
                    for so in range(S):
                        si = so + k - 1
                        if 0 <= si < S:
                            m[ci * S + si, co * S + so] = wc[co, ci, k]
        return m

    w1e = eff(w1, 29, 16)                       # [232, 128]
    w1b = np.zeros((105, 128), np.float32)
    w1b[0:104] = w1e[128:232]
    w1b[104] = np.repeat(b1, S)                 # ones-row bias
    w2e = eff(w2, 16, 8)                        # [128, 64]
    w3e = eff(w3, 8, 4)                         # [64, 32]

    # conv4 stationaries: one [33, 128] per output s; row 32 = bias.
    w4s = np.zeros((33, 8, 128), np.float32)
    for s in range(S):
        for c3 in range(4):
            for s3 in range(S):
                k = s3 - s + 1
                if 0 <= k < 3:
                    w4s[c3 * S + s3, s, :] = w4[:, c3, k]
    w4s[32, :, :] = b4[None, :]

    return dict(
        w1a=w1e[:128].astype(np.float16),
        w1b=w1b.astype(np.float16),
        w2e=w2e.astype(np.float16),
        w3e=w3e.astype(np.float16),
        w4s=np.ascontiguousarray(w4s.reshape(33, 8 * 128)).astype(np.float16),
        wlt=np.ascontiguousarray(wl.T).astype(np.float16),
        b2v=np.repeat(b2, S).reshape(64, 1).astype(np.float32),
        b3v=np.repeat(b3, S).reshape(32, 1).astype(np.float32),
    )


def _host_x(x):
    # [B, C, S] f32 -> transposed f16 [(c s)+ones, B]
    xt = np.empty((XROWS, B), np.float16)
    xt[0:CS] = np.asarray(x, np.float32).reshape(B, CS).T.astype(np.float16)
    xt[CS] = 1.0
    return xt


_NC_CACHE = None


def kernel(x, w1, b1, w2, b2, w3, b3, w4, b4, wl, bl):
    global _NC_CACHE
    xt = _host_x(x)
    wmap = _host_weights(
        np.asarray(w1, np.float32), np.asarray(b1, np.float32),
        np.asarray(w2, np.float32), np.asarray(b2, np.float32),
        np.asarray(w3, np.float32), np.asarray(b3, np.float32),
        np.asarray(w4, np.float32), np.asarray(b4, np.float32),
        np.asarray(wl, np.float32))
    # bl is constant along the softmax axis -> cancels; intentionally unused.

    if _NC_CACHE is None:
        _NC_CACHE = _build_nc()
    nc = _NC_CACHE

    core_ids = list(range(NCORES))
    in_maps = []
    for i in core_ids:
        m = {"xt": np.ascontiguousarray(xt[:, i * BPC:(i + 1) * BPC])}
        m.update(wmap)
        in_maps.append(m)
    res = run_bass_kernel_spmd(nc, in_maps, core_ids)
    outs = [res.results[i]["out"].T for i in range(NCORES)]
    return np.concatenate(outs, axis=0).astype(np.float32)
